# revision 6
# baseline (speedup 1.0000x reference)
"""Self-contained Trainium2 Bass kernel for the Sobel magnitude-gradient-error
loss (nn_MGE): mean(|sqrt-diff of Sobel magnitudes|) over [64,1,512,512] pairs.

Distribution: pure data-parallel, batch 64 split as 8 images per NeuronCore;
each core emits per-partition partial sums [128, 8]; host reduces to the mean.
"""

import sys
import types

sys.path.insert(0, "/opt/trn_rl_repo")

import numpy as np

# ---------------------------------------------------------------- axon NTFF
# The container's antenv stub lacks axon_hooks; install it so trace=True can
# register the NTFF profiling hook (used by test.py; harmless otherwise).
if "antenv.axon_hooks" not in sys.modules:
    _m = types.ModuleType("antenv.axon_hooks")
    _m._h = None
    _m.set_axon_ntff_profile_hook = lambda h: setattr(_m, "_h", h)
    _m.get_axon_ntff_profile_hook = lambda: _m._h
    sys.modules["antenv.axon_hooks"] = _m
    try:
        import antenv

        antenv.axon_hooks = _m
    except Exception:
        pass

import concourse.bass as bass
import concourse.tile as tile
from concourse import bacc, mybir
import concourse.bass_utils as bass_utils
import concourse.dve_ops as dve_ops
from concourse.dve_ops import DveOp, OPS
from concourse.dve_spec import Spec, Src0, Src1, C0, Zero, sq, maxx, lower, AluOp
from concourse.dve_uop import DveOpSpec

# uploads need bucket creds the container doesn't have; traces stay local
bass_utils.upload_artifacts = lambda tmpdir: "local://skipped"

N_CORES = 8
PAIRS_PER_CORE = 8
H = W = 512
NBLK = 4  # 4 row-blocks of 128
FP32 = mybir.dt.float32
BF16 = mybir.dt.bfloat16


def _register_op(name, spec, subdim=False):
    for op in OPS:
        if op.name == name:
            return op
    shas = {}
    for ver in ("v3", "v4"):
        tmp = DveOpSpec(name=name, opcode=0, uops=lower(spec, ver=ver), rd1_en=True)
        shas[ver] = tmp.sha(ver)
    op = DveOp(name, spec, subdim, uops_sha=shas)
    OPS.append(op)
    dve_ops.CUSTOM_DVE_SPECS[name] = spec
    dve_ops._SUB_OPCODE_FOR_NAME[name] = dve_ops._CUSTOM_DVE_ROW_BASE + len(OPS) - 1
    return op


# out = in0^2 + in1  (in0 = PSUM conv output, in1 = pre-squared other conv;
# the ISA allows only one PSUM stream per DVE op, so ScalarE squares the other)
SQADD1 = _register_op(
    "SQADD1_ANT",
    Spec(
        body=sq(Src0) + Src1,
        reference=lambda in0, in1, s0, s1, imm2: in0 * in0 + in1,
    ),
)

# out = |in0 - in1| ; accum_out = s0 + sum(out)  (final loss reduction)
def _absdiff_ref(in0, in1, s0, s1, imm2):
    b = np.abs(in0.astype(np.float32) - in1.astype(np.float32))
    return b, s0 + b.reshape(b.shape[0], -1).sum(axis=-1, keepdims=True)


_d = Src0 - Src1
ABSDIFF = _register_op(
    "ABSDIFF_ACC_ANT",
    Spec(
        body=maxx(_d, Zero - _d),
        accum=AluOp.ADD,
        accum_init=C0,
        reference=_absdiff_ref,
    ),
)


def _band_matrices():
    """lhsT conv matrices: out[m] = sum_k lhsT[k, m] * rhs[k]."""
    d = np.zeros((128, 128), np.float32)
    s = np.zeros((128, 128), np.float32)
    for m in range(128):
        if m + 1 <= 127:
            d[m + 1, m] += 1.0
        if m - 1 >= 0:
            d[m - 1, m] -= 1.0
        s[m, m] += 2.0
        if m + 1 <= 127:
            s[m + 1, m] += 1.0
        if m - 1 >= 0:
            s[m - 1, m] += 1.0
    hn = np.zeros((128, 128), np.float32)
    hn[0, 127] = 1.0  # next block's row 0 -> out row 127 (coeff +1)
    hp_pos = np.zeros((128, 128), np.float32)
    hp_pos[127, 0] = 1.0  # prev block's row 127 -> out row 0 (S conv)
    hp_neg = np.zeros((128, 128), np.float32)
    hp_neg[127, 0] = -1.0  # prev block's row 127 -> out row 0 (D conv)
    import ml_dtypes

    return {
        "dband": d.astype(ml_dtypes.bfloat16),
        "sband": s.astype(ml_dtypes.bfloat16),
        "hn": hn.astype(ml_dtypes.bfloat16),
        "hp_pos": hp_pos.astype(ml_dtypes.bfloat16),
        "hp_neg": hp_neg.astype(ml_dtypes.bfloat16),
    }


def build(n_pairs=PAIRS_PER_CORE):
    nc = bacc.Bacc(None, target_bir_lowering=False, debug=False)

    yp = nc.dram_tensor("y_p", [n_pairs, H, W], FP32, kind="ExternalInput")
    yt = nc.dram_tensor("y_t", [n_pairs, H, W], FP32, kind="ExternalInput")
    consts = {
        n: nc.dram_tensor(n, [128, 128], BF16, kind="ExternalInput")
        for n in ("dband", "sband", "hn", "hp_pos", "hp_neg")
    }
    out = nc.dram_tensor("out", [128, n_pairs], FP32, kind="ExternalOutput")

    with tile.TileContext(nc) as tc:
        with (
            tc.tile_pool(name="cst", bufs=1) as cst,
            tc.tile_pool(name="xp", bufs=3) as xp,
            tc.tile_pool(name="pp", bufs=2) as pp,
            tc.tile_pool(name="ab", bufs=4) as ab,
            tc.tile_pool(name="m2p", bufs=2) as m2p,
            tc.tile_pool(name="magp", bufs=3) as magp,
            tc.tile_pool(name="absp", bufs=2) as absp,
            tc.tile_pool(name="accp", bufs=1) as accp,
            tc.tile_pool(name="psp", bufs=3, space="PSUM") as psp,
        ):
            cmat = {}
            for n, t in consts.items():
                cmat[n] = cst.tile([128, 128], BF16, name="cst_" + n, tag="cst_" + n)
                nc.sync.dma_start(cmat[n][:], t[:])

            acc = accp.tile([128, n_pairs], FP32)

            for b in range(n_pairs):
                mags = []
                for src in (yp, yt):
                    img = src[b].rearrange("(c p) w -> p c w", p=128)
                    x = xp.tile([128, NBLK, W], FP32, tag="x")
                    nc.sync.dma_start(x[:], img)

                    # p[j] = x[j] + x[j+1]  (fp32 in -> bf16 out)
                    p = pp.tile([128, NBLK, W], BF16, tag="p")
                    nc.vector.tensor_add(
                        p[:, :, 0:511], x[:, :, 0:511], x[:, :, 1:512]
                    )
                    # A = [1,2,1] smoothing along wcols; B = [-1,0,1] derivative
                    A = ab.tile([128, NBLK, W], BF16, tag="A")
                    nc.vector.tensor_add(
                        A[:, :, 1:511], p[:, :, 0:510], p[:, :, 1:511]
                    )
                    nc.vector.tensor_add(A[:, :, 0:1], p[:, :, 0:1], x[:, :, 0:1])
                    nc.vector.tensor_add(
                        A[:, :, 511:512], p[:, :, 510:511], x[:, :, 511:512]
                    )
                    B = ab.tile([128, NBLK, W], BF16, tag="B")
                    nc.vector.tensor_tensor(
                        B[:, :, 1:511],
                        p[:, :, 1:511],
                        p[:, :, 0:510],
                        mybir.AluOpType.subtract,
                    )
                    nc.vector.tensor_copy(B[:, :, 0:1], x[:, :, 1:2])
                    nc.vector.tensor_scalar_mul(
                        B[:, :, 511:512], x[:, :, 510:511], -1.0
                    )

                    m2 = m2p.tile([128, NBLK, W], FP32, tag="m2")
                    sqh = pp.tile([128, NBLK, W], BF16, tag="sqh")
                    for blk in range(NBLK):
                        gh = psp.tile([128, W], FP32, tag="gh")
                        mm = [(cmat["dband"], A[:, blk, :])]
                        if blk > 0:
                            mm.append((cmat["hp_neg"], A[:, blk - 1, :]))
                        if blk < NBLK - 1:
                            mm.append((cmat["hn"], A[:, blk + 1, :]))
                        for i, (w_, rhs) in enumerate(mm):
                            nc.tensor.matmul(
                                gh[:],
                                w_[:],
                                rhs,
                                start=(i == 0),
                                stop=(i == len(mm) - 1),
                            )
                        gv = psp.tile([128, W], FP32, tag="gv")
                        mm = [(cmat["sband"], B[:, blk, :])]
                        if blk > 0:
                            mm.append((cmat["hp_pos"], B[:, blk - 1, :]))
                        if blk < NBLK - 1:
                            mm.append((cmat["hn"], B[:, blk + 1, :]))
                        for i, (w_, rhs) in enumerate(mm):
                            nc.tensor.matmul(
                                gv[:],
                                w_[:],
                                rhs,
                                start=(i == 0),
                                stop=(i == len(mm) - 1),
                            )
                        nc.scalar.square(sqh[:, blk, :], gh[:])
                        nc.vector._custom_dve(
                            SQADD1, out=m2[:, blk, :], in0=gv[:], in1=sqh[:, blk, :]
                        )

                    mag = magp.tile([128, NBLK, W], BF16, tag="mag")
                    nc.scalar.activation(
                        mag[:], m2[:], mybir.ActivationFunctionType.Sqrt
                    )
                    mags.append(mag)

                scratch = absp.tile([128, NBLK, W], BF16, tag="scr")
                nc.vector._custom_dve(
                    ABSDIFF,
                    out=scratch[:],
                    in0=mags[1][:],
                    in1=mags[0][:],
                    s0=0.0,
                    accum_out=acc[:, b : b + 1],
                )

            nc.sync.dma_start(out[:], acc[:])

    nc.compile()
    return nc


_CACHED = {}


def _get_nc(n_pairs=PAIRS_PER_CORE):
    if n_pairs not in _CACHED:
        _CACHED[n_pairs] = build(n_pairs)
    return _CACHED[n_pairs]


def kernel(y_p: np.ndarray, y_t: np.ndarray) -> np.ndarray:
    assert y_p.shape == (64, 1, H, W) and y_t.shape == (64, 1, H, W)
    ypf = np.ascontiguousarray(y_p.reshape(64, H, W), dtype=np.float32)
    ytf = np.ascontiguousarray(y_t.reshape(64, H, W), dtype=np.float32)
    consts = _band_matrices()

    nc = _get_nc()
    in_maps = []
    for c in range(N_CORES):
        s = slice(c * PAIRS_PER_CORE, (c + 1) * PAIRS_PER_CORE)
        in_maps.append({"y_p": ypf[s], "y_t": ytf[s], **consts})

    res = bass_utils.run_bass_kernel_spmd(nc, in_maps, core_ids=list(range(N_CORES)))
    total = np.float64(0.0)
    for r in res.results:
        total += np.sum(r["out"].astype(np.float64))
    mean = total / float(64 * H * W)
    return np.float32(mean)


# revision 7
# speedup vs baseline: 1.2710x; 1.2710x over previous
"""Self-contained Trainium2 Bass kernel for the Sobel magnitude-gradient-error
loss (nn_MGE): mean(|sqrt-diff of Sobel magnitudes|) over [64,1,512,512] pairs.

Distribution: pure data-parallel, batch 64 split as 8 images per NeuronCore;
each core emits per-partition partial sums [128, 8]; host reduces to the mean.

Per-image pipeline (x shipped as bf16, zero-padded to 514 cols in SBUF):
  DVE : p = x[j] + x[j+1]            (bf16 2x, one op incl. borders)
        A = p[j] + p[j+1]            ([1,2,1] col smoothing)
        B = p[j+1] - p[j]            ([-1,0,1] col derivative)
  PE  : gh = Dband @ A, gv = Sband @ B   (row convs; 128-row blocks with
        full-K halo matmuls accumulating cross-block rows into PSUM)
  ACT : sqh = Square(gh)             (PSUM -> SBUF bf16)
  DVE : m2 = gv^2 + sqh              (custom op, one PSUM stream)
  ACT : mag = Sqrt(m2)
  DVE : |mag_t - mag_p| + per-partition running sum (custom op)
"""

import sys
import types

sys.path.insert(0, "/opt/trn_rl_repo")

import numpy as np

# ---------------------------------------------------------------- axon NTFF
# The container's antenv stub lacks axon_hooks; install it so trace=True can
# register the NTFF profiling hook (used by test.py; harmless otherwise).
if "antenv.axon_hooks" not in sys.modules:
    _m = types.ModuleType("antenv.axon_hooks")
    _m._h = None
    _m.set_axon_ntff_profile_hook = lambda h: setattr(_m, "_h", h)
    _m.get_axon_ntff_profile_hook = lambda: _m._h
    sys.modules["antenv.axon_hooks"] = _m
    try:
        import antenv

        antenv.axon_hooks = _m
    except Exception:
        pass

import ml_dtypes
import concourse.bass as bass
import concourse.tile as tile
from concourse import bacc, mybir
import concourse.bass_utils as bass_utils
import concourse.dve_ops as dve_ops
from concourse.dve_ops import DveOp, OPS
from concourse.dve_spec import Spec, Src0, Src1, C0, Zero, sq, maxx, lower, AluOp
from concourse.dve_uop import DveOpSpec

# uploads need bucket creds the container doesn't have; traces stay local
bass_utils.upload_artifacts = lambda tmpdir: "local://skipped"

N_CORES = 8
PAIRS_PER_CORE = 8
H = W = 512
NBLK = 4  # 4 row-blocks of 128
WP = W + 2  # zero-padded column count
FP32 = mybir.dt.float32
BF16 = mybir.dt.bfloat16


def _register_op(name, spec, subdim=False, perf_en=None):
    for op in OPS:
        if op.name == name:
            return op
    shas = {}
    for ver in ("v3", "v4"):
        tmp = DveOpSpec(name=name, opcode=0, uops=lower(spec, ver=ver), rd1_en=True)
        shas[ver] = tmp.sha(ver)
    op = DveOp(name, spec, subdim, uops_sha=shas, perf_en=perf_en or {})
    OPS.append(op)
    dve_ops.CUSTOM_DVE_SPECS[name] = spec
    dve_ops._SUB_OPCODE_FOR_NAME[name] = dve_ops._CUSTOM_DVE_ROW_BASE + len(OPS) - 1
    return op


# out = in0^2 + in1  (in0 = PSUM conv output, in1 = pre-squared other conv;
# the ISA allows only one PSUM stream per DVE op, so ScalarE squares the other)
SQADD1 = _register_op(
    "SQADD1_ANT",
    Spec(
        body=sq(Src0) + Src1,
        reference=lambda in0, in1, s0, s1, imm2: in0 * in0 + in1,
    ),
)


# out = |in0 - in1| ; accum_out = s0 + sum(out)  (final loss reduction)
def _absdiff_ref(in0, in1, s0, s1, imm2):
    b = np.abs(in0.astype(np.float32) - in1.astype(np.float32))
    return b, s0 + b.reshape(b.shape[0], -1).sum(axis=-1, keepdims=True)


_d = Src0 - Src1
ABSDIFF = _register_op(
    "ABSDIFF_ACC_ANT",
    Spec(
        body=maxx(_d, Zero - _d),
        accum=AluOp.ADD,
        accum_init=C0,
        reference=_absdiff_ref,
    ),
)


def _band_matrices():
    """lhsT conv matrices: out[m] = sum_k lhsT[k, m] * rhs[k]."""
    d = np.zeros((128, 128), np.float32)
    s = np.zeros((128, 128), np.float32)
    for m in range(128):
        if m + 1 <= 127:
            d[m + 1, m] += 1.0
        if m - 1 >= 0:
            d[m - 1, m] -= 1.0
        s[m, m] += 2.0
        if m + 1 <= 127:
            s[m + 1, m] += 1.0
        if m - 1 >= 0:
            s[m - 1, m] += 1.0
    hn = np.zeros((128, 128), np.float32)
    hn[0, 127] = 1.0  # next block's row 0 -> out row 127 (coeff +1)
    hp_pos = np.zeros((128, 128), np.float32)
    hp_pos[127, 0] = 1.0  # prev block's row 127 -> out row 0 (S conv)
    hp_neg = np.zeros((128, 128), np.float32)
    hp_neg[127, 0] = -1.0  # prev block's row 127 -> out row 0 (D conv)
    return {
        "dband": d.astype(ml_dtypes.bfloat16),
        "sband": s.astype(ml_dtypes.bfloat16),
        "hn": hn.astype(ml_dtypes.bfloat16),
        "hp_pos": hp_pos.astype(ml_dtypes.bfloat16),
        "hp_neg": hp_neg.astype(ml_dtypes.bfloat16),
    }


def build(n_pairs=PAIRS_PER_CORE):
    nc = bacc.Bacc(None, target_bir_lowering=False, debug=False)

    yp = nc.dram_tensor("y_p", [n_pairs, H, W], BF16, kind="ExternalInput")
    yt = nc.dram_tensor("y_t", [n_pairs, H, W], BF16, kind="ExternalInput")
    consts = {
        n: nc.dram_tensor(n, [128, 128], BF16, kind="ExternalInput")
        for n in ("dband", "sband", "hn", "hp_pos", "hp_neg")
    }
    out = nc.dram_tensor("out", [128, n_pairs], FP32, kind="ExternalOutput")

    with tile.TileContext(nc) as tc:
        with (
            tc.tile_pool(name="cst", bufs=1) as cst,
            tc.tile_pool(name="xp", bufs=3) as xp,
            tc.tile_pool(name="pp", bufs=2) as pp,
            tc.tile_pool(name="ab", bufs=3) as ab,
            tc.tile_pool(name="sqp", bufs=2) as sqp,
            tc.tile_pool(name="m2p", bufs=2) as m2p,
            tc.tile_pool(name="magp", bufs=3) as magp,
            tc.tile_pool(name="absp", bufs=2) as absp,
            tc.tile_pool(name="accp", bufs=1) as accp,
            tc.tile_pool(name="psp", bufs=2, space="PSUM") as psp,
        ):
            cmat = {}
            for n, t in consts.items():
                cmat[n] = cst.tile([128, 128], BF16, name="cst_" + n, tag="cst_" + n)
                nc.sync.dma_start(cmat[n][:], t[:])

            acc = accp.tile([128, n_pairs], FP32)

            for b in range(n_pairs):
                mags = []
                for src in (yp, yt):
                    img = src[b].rearrange("(c p) w -> p c w", p=128)
                    x = xp.tile([128, NBLK, WP], BF16, tag="x")
                    nc.gpsimd.memset(x[:, :, 0:1], 0.0)
                    nc.gpsimd.memset(x[:, :, W + 1 : W + 2], 0.0)
                    nc.sync.dma_start(x[:, :, 1 : W + 1], img)

                    # p[j] = xpad[j] + xpad[j+1], j = 0..512  (bf16 2x)
                    p = pp.tile([128, NBLK, W + 1], BF16, tag="p")
                    nc.vector.tensor_add(p[:], x[:, :, 0 : W + 1], x[:, :, 1 : W + 2])
                    # A = [1,2,1] smoothing; B = [-1,0,1] derivative (full width)
                    A = ab.tile([128, NBLK, W], BF16, tag="A")
                    nc.vector.tensor_add(A[:], p[:, :, 0:W], p[:, :, 1 : W + 1])
                    B = ab.tile([128, NBLK, W], BF16, tag="B")
                    nc.vector.tensor_tensor(
                        B[:], p[:, :, 1 : W + 1], p[:, :, 0:W], mybir.AluOpType.subtract
                    )

                    m2 = m2p.tile([128, NBLK, W], FP32, tag="m2")
                    sqh = sqp.tile([128, NBLK, W], BF16, tag="sqh")
                    m2v = m2.rearrange("q (h u) w -> q h (u w)", u=2)
                    sqhv = sqh.rearrange("q (h u) w -> q h (u w)", u=2)
                    for half in range(NBLK // 2):
                        gh = psp.tile([128, 2 * W], FP32, tag="gh")
                        gv = psp.tile([128, 2 * W], FP32, tag="gv")
                        for u in range(2):
                            blk = 2 * half + u
                            mm = [(cmat["dband"], A[:, blk, :])]
                            if blk > 0:
                                mm.append((cmat["hp_neg"], A[:, blk - 1, :]))
                            if blk < NBLK - 1:
                                mm.append((cmat["hn"], A[:, blk + 1, :]))
                            for i, (w_, rhs) in enumerate(mm):
                                nc.tensor.matmul(
                                    gh[:, u * W : (u + 1) * W],
                                    w_[:],
                                    rhs,
                                    start=(i == 0),
                                    stop=(i == len(mm) - 1),
                                )
                            mm = [(cmat["sband"], B[:, blk, :])]
                            if blk > 0:
                                mm.append((cmat["hp_pos"], B[:, blk - 1, :]))
                            if blk < NBLK - 1:
                                mm.append((cmat["hn"], B[:, blk + 1, :]))
                            for i, (w_, rhs) in enumerate(mm):
                                nc.tensor.matmul(
                                    gv[:, u * W : (u + 1) * W],
                                    w_[:],
                                    rhs,
                                    start=(i == 0),
                                    stop=(i == len(mm) - 1),
                                )
                        nc.scalar.square(sqhv[:, half, :], gh[:])
                        nc.vector._custom_dve(
                            SQADD1, out=m2v[:, half, :], in0=gv[:], in1=sqhv[:, half, :]
                        )

                    mag = magp.tile([128, NBLK, W], BF16, tag="mag")
                    nc.scalar.activation(
                        mag[:], m2[:], mybir.ActivationFunctionType.Sqrt
                    )
                    mags.append(mag)

                scratch = absp.tile([128, NBLK, W], BF16, tag="scr")
                nc.vector._custom_dve(
                    ABSDIFF,
                    out=scratch[:],
                    in0=mags[1][:],
                    in1=mags[0][:],
                    s0=0.0,
                    accum_out=acc[:, b : b + 1],
                )

            nc.sync.dma_start(out[:], acc[:])

    nc.compile()
    return nc


_CACHED = {}


def _get_nc(n_pairs=PAIRS_PER_CORE):
    if n_pairs not in _CACHED:
        _CACHED[n_pairs] = build(n_pairs)
    return _CACHED[n_pairs]


def _to_bf16(a):
    return np.ascontiguousarray(a.astype(ml_dtypes.bfloat16))


def kernel(y_p: np.ndarray, y_t: np.ndarray) -> np.ndarray:
    assert y_p.shape == (64, 1, H, W) and y_t.shape == (64, 1, H, W)
    ypf = _to_bf16(np.asarray(y_p).reshape(64, H, W))
    ytf = _to_bf16(np.asarray(y_t).reshape(64, H, W))
    consts = _band_matrices()

    nc = _get_nc()
    in_maps = []
    for c in range(N_CORES):
        s = slice(c * PAIRS_PER_CORE, (c + 1) * PAIRS_PER_CORE)
        in_maps.append({"y_p": ypf[s], "y_t": ytf[s], **consts})

    res = bass_utils.run_bass_kernel_spmd(nc, in_maps, core_ids=list(range(N_CORES)))
    total = np.float64(0.0)
    for r in res.results:
        total += np.sum(r["out"].astype(np.float64))
    mean = total / float(64 * H * W)
    return np.float32(mean)


# revision 11
# speedup vs baseline: 1.3525x; 1.0642x over previous
"""Self-contained Trainium2 Bass kernel for the Sobel magnitude-gradient-error
loss (nn_MGE): mean(|sqrt-diff of Sobel magnitudes|) over [64,1,512,512] pairs.

Distribution: pure data-parallel, batch 64 split as 8 images per NeuronCore;
each core emits per-partition partial sums [128, 8]; host reduces to the mean.

Per-image pipeline (x shipped as bf16, zero-padded to 514 cols in SBUF):
  DVE : p = x[j] + x[j+1]            (bf16 2x, one op incl. borders)
        A = p[j] + p[j+1]            ([1,2,1] col smoothing)
        B = p[j+1] - p[j]            ([-1,0,1] col derivative)
  PE  : gh = Dband @ A, gv = Sband @ B   (row convs; 128-row blocks with
        full-K halo matmuls accumulating cross-block rows into PSUM)
  ACT : sqh = Square(gh)             (PSUM -> SBUF bf16)
  DVE : m2 = gv^2 + sqh              (custom op, one PSUM stream)
  ACT : mag = Sqrt(m2)
  DVE : |mag_t - mag_p| + per-partition running sum (custom op)
"""

import sys
import types

sys.path.insert(0, "/opt/trn_rl_repo")

import numpy as np

# ---------------------------------------------------------------- axon NTFF
# The container's antenv stub lacks axon_hooks; install it so trace=True can
# register the NTFF profiling hook (used by test.py; harmless otherwise).
if "antenv.axon_hooks" not in sys.modules:
    _m = types.ModuleType("antenv.axon_hooks")
    _m._h = None
    _m.set_axon_ntff_profile_hook = lambda h: setattr(_m, "_h", h)
    _m.get_axon_ntff_profile_hook = lambda: _m._h
    sys.modules["antenv.axon_hooks"] = _m
    try:
        import antenv

        antenv.axon_hooks = _m
    except Exception:
        pass

import ml_dtypes
import concourse.bass as bass
import concourse.tile as tile
from concourse import bacc, mybir
import concourse.bass_utils as bass_utils
import concourse.dve_ops as dve_ops
from concourse.dve_ops import DveOp, OPS
from concourse.dve_spec import Spec, Src0, Src1, C0, Zero, sq, maxx, lower, AluOp
from concourse.dve_uop import DveOpSpec

# uploads need bucket creds the container doesn't have; traces stay local
bass_utils.upload_artifacts = lambda tmpdir: "local://skipped"

N_CORES = 8
PAIRS_PER_CORE = 8
H = W = 512
NBLK = 4  # 4 row-blocks of 128
WP = W + 2  # zero-padded column count
FP32 = mybir.dt.float32
BF16 = mybir.dt.bfloat16


def _register_op(name, spec, subdim=False, perf_en=None):
    for op in OPS:
        if op.name == name:
            return op
    shas = {}
    for ver in ("v3", "v4"):
        tmp = DveOpSpec(name=name, opcode=0, uops=lower(spec, ver=ver), rd1_en=True)
        shas[ver] = tmp.sha(ver)
    op = DveOp(name, spec, subdim, uops_sha=shas, perf_en=perf_en or {})
    OPS.append(op)
    dve_ops.CUSTOM_DVE_SPECS[name] = spec
    dve_ops._SUB_OPCODE_FOR_NAME[name] = dve_ops._CUSTOM_DVE_ROW_BASE + len(OPS) - 1
    return op


# out = in0^2 + in1  (in0 = PSUM conv output, in1 = pre-squared other conv;
# the ISA allows only one PSUM stream per DVE op, so ScalarE squares the other)
SQADD1 = _register_op(
    "SQADD1_ANT",
    Spec(
        body=sq(Src0) + Src1,
        reference=lambda in0, in1, s0, s1, imm2: in0 * in0 + in1,
    ),
)


# out = |in0 - in1| ; accum_out = s0 + sum(out)  (final loss reduction)
def _absdiff_ref(in0, in1, s0, s1, imm2):
    b = np.abs(in0.astype(np.float32) - in1.astype(np.float32))
    return b, s0 + b.reshape(b.shape[0], -1).sum(axis=-1, keepdims=True)


_d = Src0 - Src1
ABSDIFF = _register_op(
    "ABSDIFF_ACC_ANT",
    Spec(
        body=maxx(_d, Zero - _d),
        accum=AluOp.ADD,
        accum_init=C0,
        reference=_absdiff_ref,
    ),
    perf_en={"v3": True, "v4": True},
)


def _band_matrices():
    """lhsT conv matrices: out[m] = sum_k lhsT[k, m] * rhs[k]."""
    d = np.zeros((128, 128), np.float32)
    s = np.zeros((128, 128), np.float32)
    for m in range(128):
        if m + 1 <= 127:
            d[m + 1, m] += 1.0
        if m - 1 >= 0:
            d[m - 1, m] -= 1.0
        s[m, m] += 2.0
        if m + 1 <= 127:
            s[m + 1, m] += 1.0
        if m - 1 >= 0:
            s[m - 1, m] += 1.0
    hn = np.zeros((128, 128), np.float32)
    hn[0, 127] = 1.0  # next block's row 0 -> out row 127 (coeff +1)
    hp_pos = np.zeros((128, 128), np.float32)
    hp_pos[127, 0] = 1.0  # prev block's row 127 -> out row 0 (S conv)
    hp_neg = np.zeros((128, 128), np.float32)
    hp_neg[127, 0] = -1.0  # prev block's row 127 -> out row 0 (D conv)
    return {
        "dband": d.astype(ml_dtypes.bfloat16),
        "sband": s.astype(ml_dtypes.bfloat16),
        "hn": hn.astype(ml_dtypes.bfloat16),
        "hp_pos": hp_pos.astype(ml_dtypes.bfloat16),
        "hp_neg": hp_neg.astype(ml_dtypes.bfloat16),
    }


def _ratio_gen(frac):
    acc = 0.0
    while True:
        acc += frac
        if acc >= 1.0 - 1e-9:
            acc -= 1.0
            yield True
        else:
            yield False


def build(n_pairs=PAIRS_PER_CORE, act_frac=0.5):
    act_split = _ratio_gen(act_frac)
    nc = bacc.Bacc(None, target_bir_lowering=False, debug=False)

    yp = nc.dram_tensor("y_p", [n_pairs, H, W], BF16, kind="ExternalInput")
    yt = nc.dram_tensor("y_t", [n_pairs, H, W], BF16, kind="ExternalInput")
    consts = {
        n: nc.dram_tensor(n, [128, 128], BF16, kind="ExternalInput")
        for n in ("dband", "sband", "hn", "hp_pos", "hp_neg")
    }
    out = nc.dram_tensor("out", [128, n_pairs], FP32, kind="ExternalOutput")

    with tile.TileContext(nc) as tc:
        with (
            tc.tile_pool(name="cst", bufs=1) as cst,
            tc.tile_pool(name="xp", bufs=3) as xp,
            tc.tile_pool(name="pp", bufs=2) as pp,
            tc.tile_pool(name="ab", bufs=3) as ab,
            tc.tile_pool(name="sqp", bufs=2) as sqp,
            tc.tile_pool(name="m2p", bufs=2) as m2p,
            tc.tile_pool(name="magp", bufs=3) as magp,
            tc.tile_pool(name="absp", bufs=2) as absp,
            tc.tile_pool(name="accp", bufs=1) as accp,
            tc.tile_pool(name="psp", bufs=2, space="PSUM") as psp,
        ):
            cmat = {}
            for n, t in consts.items():
                cmat[n] = cst.tile([128, 128], BF16, name="cst_" + n, tag="cst_" + n)
                nc.sync.dma_start(cmat[n][:], t[:])

            acc = accp.tile([128, n_pairs], FP32)

            for b in range(n_pairs):
                mags = []
                for src in (yp, yt):
                    img = src[b].rearrange("(c p) w -> p c w", p=128)
                    x = xp.tile([128, NBLK, WP], BF16, tag="x")
                    nc.gpsimd.memset(x[:, :, 0:1], 0.0)
                    nc.gpsimd.memset(x[:, :, W + 1 : W + 2], 0.0)
                    nc.sync.dma_start(x[:, :, 1 : W + 1], img)

                    # p[j] = xpad[j] + xpad[j+1], j = 0..512  (bf16 2x)
                    p = pp.tile([128, NBLK, W + 1], BF16, tag="p")
                    nc.vector.tensor_add(p[:], x[:, :, 0 : W + 1], x[:, :, 1 : W + 2])
                    # A = [1,2,1] smoothing; B = [-1,0,1] derivative (full width)
                    A = ab.tile([128, NBLK, W], BF16, tag="A")
                    nc.vector.tensor_add(A[:], p[:, :, 0:W], p[:, :, 1 : W + 1])
                    B = ab.tile([128, NBLK, W], BF16, tag="B")
                    nc.vector.tensor_tensor(
                        B[:], p[:, :, 1 : W + 1], p[:, :, 0:W], mybir.AluOpType.subtract
                    )

                    m2 = m2p.tile([128, NBLK, W], BF16, tag="m2")
                    sqh = sqp.tile([128, NBLK, W], BF16, tag="sqh")
                    sqv = sqp.tile([128, NBLK, W], BF16, tag="sqv")
                    m2v = m2.rearrange("q (h u) w -> q h (u w)", u=2)
                    sqhv = sqh.rearrange("q (h u) w -> q h (u w)", u=2)
                    sqvv = sqv.rearrange("q (h u) w -> q h (u w)", u=2)
                    for half in range(NBLK // 2):
                        gh = psp.tile([128, 2 * W], FP32, tag="gh")
                        gv = psp.tile([128, 2 * W], FP32, tag="gv")
                        for u in range(2):
                            blk = 2 * half + u
                            mm = [(cmat["dband"], A[:, blk, :])]
                            if blk > 0:
                                mm.append((cmat["hp_neg"], A[:, blk - 1, :]))
                            if blk < NBLK - 1:
                                mm.append((cmat["hn"], A[:, blk + 1, :]))
                            for i, (w_, rhs) in enumerate(mm):
                                nc.tensor.matmul(
                                    gh[:, u * W : (u + 1) * W],
                                    w_[:],
                                    rhs,
                                    start=(i == 0),
                                    stop=(i == len(mm) - 1),
                                )
                            mm = [(cmat["sband"], B[:, blk, :])]
                            if blk > 0:
                                mm.append((cmat["hp_pos"], B[:, blk - 1, :]))
                            if blk < NBLK - 1:
                                mm.append((cmat["hn"], B[:, blk + 1, :]))
                            for i, (w_, rhs) in enumerate(mm):
                                nc.tensor.matmul(
                                    gv[:, u * W : (u + 1) * W],
                                    w_[:],
                                    rhs,
                                    start=(i == 0),
                                    stop=(i == len(mm) - 1),
                                )
                        nc.scalar.square(sqhv[:, half, :], gh[:])
                        if next(act_split):
                            # ScalarE squares gv too; DVE only does a 2x bf16 add
                            nc.scalar.square(sqvv[:, half, :], gv[:])
                            nc.vector.tensor_add(
                                m2v[:, half, :], sqhv[:, half, :], sqvv[:, half, :]
                            )
                        else:
                            nc.vector._custom_dve(
                                SQADD1,
                                out=m2v[:, half, :],
                                in0=gv[:],
                                in1=sqhv[:, half, :],
                            )

                    mag = magp.tile([128, NBLK, W], BF16, tag="mag")
                    nc.scalar.activation(
                        mag[:], m2[:], mybir.ActivationFunctionType.Sqrt
                    )
                    mags.append(mag)

                scratch = absp.tile([128, NBLK, W], BF16, tag="scr")
                nc.vector._custom_dve(
                    ABSDIFF,
                    out=scratch[:],
                    in0=mags[1][:],
                    in1=mags[0][:],
                    s0=0.0,
                    accum_out=acc[:, b : b + 1],
                )

            nc.sync.dma_start(out[:], acc[:])

    nc.compile()
    return nc


_CACHED = {}


def _get_nc(n_pairs=PAIRS_PER_CORE):
    if n_pairs not in _CACHED:
        _CACHED[n_pairs] = build(n_pairs)
    return _CACHED[n_pairs]


def _to_bf16(a):
    return np.ascontiguousarray(a.astype(ml_dtypes.bfloat16))


def kernel(y_p: np.ndarray, y_t: np.ndarray) -> np.ndarray:
    assert y_p.shape == (64, 1, H, W) and y_t.shape == (64, 1, H, W)
    ypf = _to_bf16(np.asarray(y_p).reshape(64, H, W))
    ytf = _to_bf16(np.asarray(y_t).reshape(64, H, W))
    consts = _band_matrices()

    nc = _get_nc()
    in_maps = []
    for c in range(N_CORES):
        s = slice(c * PAIRS_PER_CORE, (c + 1) * PAIRS_PER_CORE)
        in_maps.append({"y_p": ypf[s], "y_t": ytf[s], **consts})

    res = bass_utils.run_bass_kernel_spmd(nc, in_maps, core_ids=list(range(N_CORES)))
    total = np.float64(0.0)
    for r in res.results:
        total += np.sum(r["out"].astype(np.float64))
    mean = total / float(64 * H * W)
    return np.float32(mean)
